# revision 34
# baseline (speedup 1.0000x reference)
"""Trainium2 Bass kernel for GQA attention (B=2, S=2048, D=2048, H=16, KVH=4).

Sharding: 8 cores = (batch b in {0,1}) x (kv-group g in {0..3}).
Each core: Q/K/V projections for its 4 q-heads + 1 kv head, RoPE, causal
softmax attention, and a partial output projection over its 512 Wo rows.
Host sums the 4 partials per batch.

On-device layout notes:
- x is passed per-core pre-transposed (xT [D, S]) so the contraction dim
  (D, then head_dim, then seq-k) is always the SBUF partition dim.
- Wq/Wk columns are pre-permuted per head on host to deinterleave RoPE
  pairs (even dims -> rows 0:64, odd dims -> rows 64:128 of each head's
  Q^T/K^T block). The same permutation on Q and K preserves q.k dots.
- RoPE uses stacked cos/sin tables cc=[c;c], ss=[s;s] [128, S] so the two
  products run as full-128-partition DVE ops (4 ops/target instead of 6).
- Scores are computed transposed (S^T [k, q]); the PV matmul consumes the
  exp'd scores directly (O^T = sum_k V[k,:]^T E^T[k,:]) so the probability
  matrix is never transposed. Softmax denominators come from an extra
  matmul with an all-ones [128,128] stationary (result broadcast across
  all partitions), normalized with an approx reciprocal + multiply.
- The causal mask is folded into the scores in PSUM via a bias matmul
  (stationary -1e9*I bf16, moving 0/1 upper-tri const bf16) accumulated
  into the same bank group as the QK matmul, so the exp->PV chain has no
  DVE hop. The diagonal k-tile with only 128 live q-cols is padded to 256
  (fp32r matmuls need free size >= 256 for 1 cyc/row); the padded columns
  are fully masked so they exp to 0.
- The V path and output projection run in bf16 (vsb/et/ones, orot/wo):
  no exp amplification there, ~0.27% total error vs the 2e-2 budget,
  and it halves their SBUF/DMA footprint. Q/K stay f32r end-to-end --
  fp8/bf16 projections put ~3%/0.4% elementwise error on the scores,
  which the softmax amplifies past tolerance (measured 4e-2 with fp8).
- Phase B runs d-outer (each x tile feeds kp/vp/qp0..3 then dies) so the
  xs pool needs only ~8 rotating buffers instead of 16 live tiles.
- All score-path matmuls are fp32r (1 cyc/row at free-size >= 256; the
  128-wide diagonal k-tile is padded to 256 because <256 runs 4x slow);
  fp32r operands must be produced as fp32r, so matmul-feeding tiles/DRAM
  tensors are declared float32r (bit-identical to f32 on host).
- FLAGS selects the measured-best configuration; pipe/hoist variants are
  kept for A/B testing (both measured neutral-to-worse: the per-engine
  4-deep dependency wait queues already bypass stalled instructions, so
  explicit software pipelining adds nothing).
"""

import math

import numpy as np

B = 2
S = 2048
D = 2048
N_HEADS = 16
N_KV_HEADS = 4
HD = 128  # head dim
G = 4  # kv groups (= heads per core group)
HPC = 4  # q heads per core
EQ = HPC * HD  # 512 q-proj cols per core
THETA = 10000.0
N_CORES = 8

SC = 4  # seq chunks of 512 in projections
QC = 4  # q chunks of 512 in attention
KT = 16  # k tiles of 128
DT = 16  # d tiles of 128

W8SCALE = 64.0  # fp8 weight pre-scale (folded out via exp scale / evac scale)

# final kernel configuration (flags also exposed for A/B bisecting)
FLAGS = dict(pipe=False, biasmask=True, newrope=True, evacalt=True,
             fp8=False, hoist=False, bigdma=False, dvedp=True,
             mergecd=True, w16=True, dlag=1, pairexp=False, pooldp=True,
             dvemask=True, dacc2=False)


def _host_tables():
    """Stacked cos/sin tables [128, S] ([c;c] and [s;s]), deinterleave perm."""
    j = np.arange(HD // 2)
    inv_freq = 1.0 / THETA ** (2 * j / HD)  # [64]
    t = np.arange(S)
    ang = np.outer(inv_freq, t)  # [64, S]
    cosT = np.cos(ang).astype(np.float32)
    sinT = np.sin(ang).astype(np.float32)
    cc = np.concatenate([cosT, cosT], axis=0)  # [128, S]
    ss = np.concatenate([sinT, sinT], axis=0)
    jj = np.arange(HD)
    perm = np.where(jj < 64, 2 * jj, 2 * (jj - 64) + 1)  # new row j <- old dim perm[j]
    return cc, ss, perm


def _host_tris():
    """Mask-bias moving constants (0/1), f32 on host, bf16 on device.

    triA [128, 512]: cols 0:128 = strict upper tri (kk > qq), rest zero.
    triB [128, 256]: cols 0:128 = all ones, cols 128:256 = strict upper tri
    (for the padded r=3 diagonal tile: [256:384] fully masked, [384:512] tri).
    """
    kk = np.arange(128)[:, None]
    qq = np.arange(128)[None, :]
    tri = (kk > qq).astype(np.float32)
    triA = np.zeros((128, 512), np.float32)
    triA[:, :128] = tri
    triB = np.concatenate([np.ones((128, 128), np.float32), tri], axis=1)
    return triA, triB


def _emit_fused(nc, tc, mybir, aps, shared, evacalt=True, newrope=True,
                w16=False, dlag=2, pairexp=False, pooldp=False,
                dvemask=False, dacc2=False):
    """Fully fused single-pass emitter: projections+RoPE (B), attention (C)
    and out-projection (D) interleaved in one loop.

    Emission order: B0, B1, C0, B2, C1, B3, C2(+D0), C3(+D1), D2, D3.
    Attention for q-chunk qc is emitted right after projection chunk qc+1,
    so its matmuls fill the projection phase's DMA-starved PE gaps and the
    projection->attention seam disappears. Out-projection blocks trail
    their q-chunk by dlag chunks (default 2) so their out-DMAs don't
    contend with the x-chunk loads still streaming in.

    Score k-tiles are paired into [128,2,512] PSUM tiles (2 banks): one
    activation instruction exps two k-tiles (halves the ~185ns/op ACT
    overhead). Full (off-diagonal) tiles pair up; of the 4 diagonal tiles
    r=0..3, (r2,r3) pair via an equal-width two-span AP, r0/r1 stay single.

    PSUM budget (8 banks): tag qp = 2x[128,2,512] (proj q-head pairs, then
    score pairs), kv = 2x[128,512] (kp/vp, then dp/pout), vtr = 2x[128,512]
    (V transposes, then PV accumulators).

    w16: x and Wq/Wk/Wv in bf16 (halves the dominant x DMA). The
    projection PSUM accumulation and everything downstream (RoPE, scores)
    stays f32, so only the input quantization (~0.4%) is added.
    """
    from contextlib import ExitStack
    from collections import deque

    from concourse import bass_isa

    f32 = mybir.dt.float32
    f32r = mybir.dt.float32r
    bf16 = mybir.dt.bfloat16
    AT = mybir.ActivationFunctionType
    if w16:
        xT, wq = aps["xT16"], aps["wq16"]
        wk, wv = aps["wk16"], aps["wv16"]
        mmdt = bf16
    else:
        xT, wq = aps["xT"], aps["wq"]
        wk, wv = aps["wk"], aps["wv"]
        mmdt = f32r
    wo16, ccT, ssT, out = aps["wo16"], aps["ccT"], aps["ssT"], aps["out"]
    (ones_sb, ident, negI_bf, triA_bf, triB_bf, masks_sb, qrot, krot, vsb,
     ident_bf, masks_bf, masksB_bf) = shared
    escale = 1.0 / math.sqrt(HD)

    with ExitStack() as stk:
        mpsum = stk.enter_context(tc.tile_pool(name="mpsum", bufs=1,
                                               space="PSUM"))
        wpool = stk.enter_context(tc.tile_pool(name="wpool", bufs=1))
        cspool = stk.enter_context(tc.tile_pool(name="cspool", bufs=1))
        xs_pool = stk.enter_context(
            tc.tile_pool(name="xs", bufs=32 if w16 else 17))
        tmp_pool = stk.enter_context(tc.tile_pool(name="ropetmp", bufs=2))
        vt_pool = stk.enter_context(tc.tile_pool(name="vtstage", bufs=1))
        opool = stk.enter_context(tc.tile_pool(name="opool", bufs=1))
        wopool = stk.enter_context(tc.tile_pool(name="wopool", bufs=1))
        et_pool = stk.enter_context(
            tc.tile_pool(name="et", bufs=8 if w16 else 7))
        rd_pool = stk.enter_context(tc.tile_pool(name="rd", bufs=2))
        ostage_pool = stk.enter_context(
            tc.tile_pool(name="ostage", bufs=4 if w16 else 3))

        orot = {
            (h, c): opool.tile(
                [128, 512], bf16, tag=f"orot{h}_{c}", name=f"orot{h}_{c}"
            )
            for h in range(HPC)
            for c in range(QC)
        }
        wo_sb = wopool.tile([128, HPC, D], bf16, tag="wo")

        # ---- weight + chunk-0 x prefetch (tiny first loads so the very
        # first matmul ungates in ~2-3us) ----
        wq_sb = wpool.tile([128, DT, EQ], mmdt, tag="wq")
        wk_sb = wpool.tile([128, DT, HD], mmdt, tag="wk")
        wv_sb = wpool.tile([128, DT, HD], mmdt, tag="wv")
        xts0 = []
        xt00 = xs_pool.tile([128, 512], mmdt, tag="xs", name="xs0_0")
        nc.sync.dma_start(wk_sb[:, 0, :], wk[0:128, :])
        nc.sync.dma_start(xt00[:, 0:256], xT[0:128, 0:256])
        nc.sync.dma_start(xt00[:, 256:512], xT[0:128, 256:512])
        xts0.append(xt00)
        wkr = wk.rearrange("(t p) e -> p t e", p=128)
        wvr = wv.rearrange("(t p) e -> p t e", p=128)
        nc.sync.dma_start(wk_sb[:, 1:4, :], wkr[:, 1:4, :])
        for d in range(DT):
            if d > 0:
                xt = xs_pool.tile([128, 512], mmdt, tag="xs", name=f"xs0_{d}")
                nc.sync.dma_start(xt[:], xT[d * 128 : (d + 1) * 128, 0:512])
                xts0.append(xt)
            nc.sync.dma_start(wq_sb[:, d, :], wq[d * 128 : (d + 1) * 128, :])
            if d % 4 == 0:
                if d > 0:
                    nc.sync.dma_start(wk_sb[:, d : d + 4, :], wkr[:, d : d + 4, :])
                nc.sync.dma_start(wv_sb[:, d : d + 4, :], wvr[:, d : d + 4, :])
        cc_sb = cspool.tile([128, S], f32, tag="cc")
        ss_sb = cspool.tile([128, S], f32, tag="ss")
        vT_sb = vt_pool.tile([128, S], bf16, tag="vT")

        # ---------------- B: one projection chunk (target-outer) --------
        # Each of the 6 projection targets (K, V, Q0..Q3) accumulates alone
        # over all 16 d-tiles, then evacuates (rope on DVE / copy on ACT)
        # while the next target's matmuls run. PSUM use is 2 rotating banks
        # (tag kv) instead of 6 held for the whole chunk, so attention's
        # score tiles (tag qp) never wait on projection-bank releases, and
        # the chunk tail only trails the last matmul by one rope (~1.7us),
        # not the full 5-target rope chain (~9us).
        def emit_bchunk(sc):
            ssl = slice(sc * 512, (sc + 1) * 512)
            # cos/sin arrive per chunk (keeps the big tables out of the
            # head of the DMA queues during the chunk-0 ramp)
            nc.sync.dma_start(cc_sb[:, ssl], ccT[:, ssl])
            nc.sync.dma_start(ss_sb[:, ssl], ssT[:, ssl])
            c = cc_sb[:, ssl]
            s = ss_sb[:, ssl]

            def _rope(src, dst):
                # dst[0:64] = x0*c - x1*s ; dst[64:128] = x0*s + x1*c
                # (src stays in PSUM: the cross-half sub/add legally mix one
                # SBUF and one PSUM operand at different base partitions)
                t_cc = tmp_pool.tile([128, 512], f32, tag="tcc", name="tcc")
                nc.vector.tensor_mul(t_cc[:], src[:], c)
                nc.vector.tensor_mul(src[:], src[:], s)
                nc.vector.tensor_sub(dst[0:64, :], t_cc[0:64, :], src[64:128, :])
                nc.vector.tensor_add(dst[64:128, :], src[0:64, :], t_cc[64:128, :])

            if sc == 0:
                # chunk 0 runs d-outer: while x tiles trickle in on a cold
                # DMA pipe, each arriving tile immediately feeds all 6
                # targets (6 matmuls per tile keeps the PE fed at ~3x the
                # DMA rate); bank decoupling doesn't matter yet because no
                # attention work exists to compete for PSUM.
                kp = mpsum.tile([128, 512], f32, tag="vtr", bufs=2,
                                name="kp")
                vp = mpsum.tile([128, 512], f32, tag="vtr", bufs=2,
                                name="vp")
                if pairexp:
                    qpp = [
                        mpsum.tile([128, 2, 512], f32, tag="qp", bufs=2,
                                   name=f"qpp{j}")
                        for j in range(2)
                    ]
                    qps = [qpp[h // 2][:, h % 2, :] for h in range(HPC)]
                else:
                    qps = [
                        mpsum.tile([128, 512], f32, tag="qp", bufs=4,
                                   name=f"qp{h}")
                        for h in range(HPC)
                    ]
                for d in range(DT):
                    xt = xts0[d]
                    first = d == 0
                    last = d == DT - 1
                    if d == 0:
                        nc.tensor.matmul(kp[:, 0:256], wk_sb[:, 0, :],
                                         xt[:, 0:256], start=True, stop=False)
                        nc.tensor.matmul(kp[:, 256:512], wk_sb[:, 0, :],
                                         xt[:, 256:512], start=False,
                                         stop=False)
                    else:
                        nc.tensor.matmul(kp[:], wk_sb[:, d, :], xt[:],
                                         start=first, stop=last)
                    nc.tensor.matmul(vp[:], wv_sb[:, d, :], xt[:],
                                     start=first, stop=last)
                    for h in range(HPC):
                        nc.tensor.matmul(
                            qps[h][:], wq_sb[:, d, h * 128 : (h + 1) * 128],
                            xt[:], start=first, stop=last)
                _rope(kp, krot[0])
                nc.scalar.copy(vT_sb[:, ssl], vp[:])
                for h in range(HPC):
                    _rope(qps[h], qrot[(h, 0)])
            else:
                xts = []
                for d in range(DT):
                    xt = xs_pool.tile([128, 512], mmdt, tag="xs",
                                      name=f"xs{sc}_{d}")
                    nc.sync.dma_start(xt[:], xT[d * 128 : (d + 1) * 128, ssl])
                    xts.append(xt)

                def accumulate(target, stat_sb, hsl=None):
                    for d in range(DT):
                        stat = stat_sb[:, d, :] if hsl is None else \
                            stat_sb[:, d, hsl]
                        nc.tensor.matmul(target[:], stat, xts[d][:],
                                         start=d == 0, stop=d == DT - 1)

                kp = mpsum.tile([128, 512], f32, tag="kv", bufs=2, name="kp")
                accumulate(kp, wk_sb)
                _rope(kp, krot[sc])
                vp = mpsum.tile([128, 512], f32, tag="kv", bufs=2, name="vp")
                accumulate(vp, wv_sb)
                nc.scalar.copy(vT_sb[:, ssl], vp[:])
                for h in range(HPC):
                    qp = mpsum.tile([128, 512], f32, tag="kv", bufs=2,
                                    name=f"qp{h}")
                    accumulate(qp, wq_sb, slice(h * 128, (h + 1) * 128))
                    _rope(qp, qrot[(h, sc)])
            for t in range(4 * sc, 4 * sc + 4):
                vtp = mpsum.tile([128, 128], bf16, tag="kv", bufs=2,
                                 name="vtr")
                nc.tensor.transpose(
                    vtp[:], vT_sb[:, t * 128 : (t + 1) * 128], ident_bf[:]
                )
                nc.scalar.copy(vsb[:, t, :], vtp[:])

        # ---------------- C/D helpers ----------------
        evac_ct = [0]

        def emit_dblock(sc_, st, mc, evac="alt"):
            """One [128,512] out-proj block: seq rows st*128..+128, out cols
            mc*512..+512, contracting this core's 512 o-dims (4 heads).
            Uses the kv tag (1 bank) so it never stalls the score-tile
            rotation on the qp tag. evac="dve" keeps the copy off the
            Scalar engine for blocks that overlap the exp-saturated
            attention tail."""
            so = st % 4
            msl = slice(mc * 512, (mc + 1) * 512)
            pout = mpsum.tile([128, 512], f32, tag="kv", bufs=2, name="pout")
            for h in range(HPC):
                nc.tensor.matmul(
                    pout[:],
                    orot[(h, sc_)][:, so * 128 : (so + 1) * 128],
                    wo_sb[:, h, msl],
                    start=(h == 0),
                    stop=(h == HPC - 1),
                )
            ost = ostage_pool.tile([128, 512], f32, tag="ost", name="ost")
            if evac == "dve" or (evacalt and evac_ct[0] % 2 == 1):
                nc.vector.tensor_copy(ost[:], pout[:])
            else:
                nc.scalar.copy(ost[:], pout[:])
            evac_ct[0] += 1
            stsl = slice(st * 128, (st + 1) * 128)
            nc.sync.dma_start(out[stsl, msl], ost[:])

        dq = deque()  # pending out-proj blocks, ready once their qc is done
        acc = {}

        def _lo_of(qc, kt):
            r = kt - 4 * qc
            if r < 0:
                return 0
            if r == 3:
                return 256
            return 128 * r

        def emit_tile(h, qc, kts, nkt):
            """Score MM + bias + exp + dacc + PV for one k-tile or an
            equal-lo pair of k-tiles (one exp instruction per call)."""
            pair = len(kts) == 2
            if pairexp:
                sp2 = mpsum.tile([128, 2, 512], f32, tag="qp", bufs=2,
                                 name="sp")
                et2 = et_pool.tile([128, 2, 512], bf16, tag="et", name="et")
                sps = [sp2[:, i, :] for i in range(2)]
                ets = [et2[:, i, :] for i in range(2)]
            else:
                sps, ets = [], []
                for _ in kts:
                    sps.append(mpsum.tile([128, 512], f32, tag="qp", bufs=4,
                                          name="sp"))
                    ets.append(et_pool.tile([128, 512], bf16, tag="et",
                                            name="et"))
            los = [_lo_of(qc, kt) for kt in kts]
            for i, kt in enumerate(kts):
                r = kt - 4 * qc
                lo = los[i]
                kc, ko = divmod(kt, 4)
                diag = r >= 0
                nc.tensor.matmul(
                    sps[i][:, lo:512],
                    krot[kc][:, ko * 128 : (ko + 1) * 128],
                    qrot[(h, qc)][:, lo:512],
                    start=True,
                    stop=not (diag and not dvemask),
                )
                if diag and not dvemask:
                    if r == 3:
                        nc.tensor.matmul(
                            sps[i][:, 256:512], negI_bf[:], triB_bf[:, 0:256],
                            start=False, stop=True,
                        )
                    else:
                        nc.tensor.matmul(
                            sps[i][:, 128 * r : 128 * r + 128],
                            negI_bf[:], triA_bf[:, 0:128],
                            start=False, stop=True,
                        )
            if pairexp and pair:
                lo = los[0]
                nc.scalar.activation(
                    et2[:, :, lo:512], sp2[:, :, lo:512], AT.Exp, scale=escale)
            else:
                for i, lo in enumerate(los):
                    nc.scalar.activation(
                        ets[i][:, lo:512], sps[i][:, lo:512], AT.Exp,
                        scale=escale)
            if dvemask:
                for i, kt in enumerate(kts):
                    r = kt - 4 * qc
                    if r < 0:
                        continue
                    if r == 3:
                        nc.vector.tensor_mul(
                            ets[i][:, 256:512], ets[i][:, 256:512],
                            masksB_bf[:])
                    else:
                        nc.vector.tensor_mul(
                            ets[i][:, 128 * r : 128 * r + 128],
                            ets[i][:, 128 * r : 128 * r + 128], masks_bf[:])
            split = dacc2 and not (h == HPC - 1 and qc == QC - 1)
            if kts[0] == 0:
                op = mpsum.tile([128, 512], f32, tag="vtr", bufs=2, name="op")
                da = rd_pool.tile([128, 512], bf16, tag="dacc", name="dacc")
                db = rd_pool.tile([128, 512], bf16, tag="dacc2",
                                  name="dacc2") if split else None
                acc[(h, qc)] = (op, da, db)
            op, da, db = acc[(h, qc)]
            for i, (kt, lo) in enumerate(zip(kts, los)):
                st_ = kt == 0
                spf = kt == nkt - 1
                nc.tensor.matmul(
                    op[:, lo:512], vsb[:, kt, :], ets[i][:, lo:512],
                    start=st_, stop=spf,
                )
                # denominator partial sums: split into two independent
                # chains (even k-tiles on DVE, odd on GPSIMD) so neither
                # engine carries the whole serial accumulation; the final
                # group stays DVE-only to keep the drain un-delayed
                if st_:
                    nc.vector.tensor_copy(da[:], ets[i][:])
                elif split and kt == 1:
                    nc.gpsimd.tensor_copy(db[:], ets[i][:])
                elif split and kt % 2 == 1:
                    nc.gpsimd.tensor_add(
                        db[:, lo:512], db[:, lo:512], ets[i][:, lo:512])
                else:
                    nc.vector.tensor_add(
                        da[:, lo:512], da[:, lo:512], ets[i][:, lo:512])
                if spf:
                    if split:
                        nc.vector.tensor_add(da[:], da[:], db[:])
                    if pooldp:
                        # cross-partition sum on the (otherwise idle) GPSIMD
                        # engine instead of a PE ones-matmul
                        ds = rd_pool.tile([128, 512], f32, tag="ds",
                                          name="ds")
                        nc.gpsimd.partition_all_reduce(
                            ds[:], da[:], 128, bass_isa.ReduceOp.add)
                        dsrc = ds
                    else:
                        dp = mpsum.tile([128, 512], f32, tag="kv", bufs=2,
                                        name="dp")
                        nc.tensor.matmul(dp[:], ones_sb[:], da[:],
                                         start=True, stop=True)
                        dsrc = dp
                    rd = rd_pool.tile([128, 512], f32, tag="rd", name="rd")
                    nc.vector.reciprocal_approx_fast(rd[:], dsrc[:])
                    nc.vector.tensor_mul(orot[(h, qc)][:], op[:], rd[:])

        tile_ct = [0]

        def maybe_dblock(evac="alt", every=2):
            tile_ct[0] += 1
            if dq and tile_ct[0] % every == 0:
                emit_dblock(*dq.popleft(), evac=evac)

        def emit_cgroup(qc):
            nkt = 4 * (qc + 1)
            evac = "dve" if qc == QC - 1 else "alt"
            every = 4 if qc == QC - 1 else 2
            nfull = 4 * qc
            for h in range(HPC):
                if pairexp:
                    groups = [(2 * p, 2 * p + 1) for p in range(nfull // 2)]
                    groups += [(nfull,), (nfull + 1,),
                               (nfull + 2, nfull + 3)]
                else:
                    groups = [(kt,) for kt in range(nkt)]
                for kts in groups:
                    emit_tile(h, qc, kts, nkt)
                    maybe_dblock(evac=evac, every=every)
                    if len(kts) == 2:
                        maybe_dblock(evac=evac, every=every)

        def enqueue_d(qc):
            for st in range(4 * qc, 4 * qc + 4):
                for mc in range(4):
                    dq.append((qc, st, mc))

        # ---------------- fused schedule ----------------
        # B0, B1, C0, B2, C1, B3, C2(+D[2-dlag]), C3(+D[3-dlag]), drain rest
        for sc in range(SC):
            emit_bchunk(sc)
            if sc == 1:
                for h in range(2):
                    nc.sync.dma_start(wo_sb[:, h, :],
                                      wo16[h * 128 : (h + 1) * 128, :])
            elif sc == 2:
                for h in range(2, HPC):
                    nc.sync.dma_start(wo_sb[:, h, :],
                                      wo16[h * 128 : (h + 1) * 128, :])
            if sc >= 1:
                qc = sc - 1
                if qc - dlag >= 0:
                    enqueue_d(qc - dlag)
                emit_cgroup(qc)
        qc = QC - 1
        if qc - dlag >= 0:
            enqueue_d(qc - dlag)
        emit_cgroup(qc)
        for q in range(max(0, QC - dlag), QC):
            enqueue_d(q)
        while dq:
            emit_dblock(*dq.popleft(), evac="alt")


def _emit_once(nc, tc, mybir, aps, shared, trim=True, split=True,
               pipe=True, biasmask=True, newrope=True, evacalt=True,
               fp8=False, bigdma=False, dvedp=False,
               pools=None):
    """One full forward pass (phases B: proj+RoPE+V, C: attention, D: out-proj).

    With pools=None each phase opens/closes its own tile pools (SBUF regions
    are reused between phases, but the allocator-level reuse serializes rep
    boundaries). With a hoisted `pools` dict (created once for all reps),
    same-tag tiles rotate buffers across reps, so rep r+1's projections can
    overlap rep r's attention/out-projection.
    """
    from contextlib import ExitStack

    f32 = mybir.dt.float32
    f32r = mybir.dt.float32r
    f8 = mybir.dt.float8e4
    bf16 = mybir.dt.bfloat16
    AT = mybir.ActivationFunctionType
    DR = mybir.MatmulPerfMode.DoubleRow
    wo16, ccT, ssT, out = aps["wo16"], aps["ccT"], aps["ssT"], aps["out"]
    (ones_sb, ident, negI_bf, triA_bf, triB_bf, masks_sb, qrot, krot, vsb,
     ident_bf, masks_bf, masksB_bf) = shared
    inv_sqrt_hd = 1.0 / math.sqrt(HD)
    escale = inv_sqrt_hd / (W8SCALE * W8SCALE if fp8 else 1.0)

    mpsum_ctx = None
    if pools is None:
        mpsum_ctx = tc.tile_pool(name="mpsum", bufs=1, space="PSUM")
        mpsum = mpsum_ctx.__enter__()
    else:
        mpsum = pools["mpsum"]

    # ---------------- Phase B: projections + RoPE + V ----------------
    with ExitStack() as stkB:
        if pools is None:
            wpool = stkB.enter_context(tc.tile_pool(name="wpool", bufs=1))
            cspool = stkB.enter_context(tc.tile_pool(name="cspool", bufs=1))
            xs_pool = stkB.enter_context(tc.tile_pool(name="xs", bufs=24))
            tmp_pool = stkB.enter_context(tc.tile_pool(name="ropetmp", bufs=2))
            vt_pool = stkB.enter_context(tc.tile_pool(name="vtstage", bufs=1))
        else:
            wpool, cspool, xs_pool = pools["wpool"], pools["cspool"], pools["xs"]
            tmp_pool, vt_pool = pools["ropetmp"], pools["vtstage"]
        # interleave the first seq-chunk's activation loads with the weight
        # slices so the d=0 matmuls un-gate early on a cold start; one DMA
        # instruction = one HW queue, so splitting also parallelizes.
        # tiny first loads so the very first matmul (kp, d=0) ungates in
        # ~2-3us: wk d=0 slice and the two halves of x chunk0/d0 go first
        xts0 = []
        if fp8:
            NT = DT // 2  # DoubleRow: 2 contraction tiles per matmul
            xT8, wq8 = aps["xT8"], aps["wq8"]
            wk8, wv8 = aps["wk8"], aps["wv8"]
            wq_sb = wpool.tile([128, NT, 2, EQ], f8, tag="wq")
            wk_sb = wpool.tile([128, NT, 2, HD], f8, tag="wk")
            wv_sb = wpool.tile([128, NT, 2, HD], f8, tag="wv")
            xt00 = xs_pool.tile([128, 2, 512], f8, tag="xs", name="xs0_0")
            nc.sync.dma_start(wk_sb[:, 0], wk8[0])
            nc.sync.dma_start(xt00[:, :, 0:256], xT8[0, :, :, 0:256])
            nc.sync.dma_start(xt00[:, :, 256:512], xT8[0, :, :, 256:512])
            xts0.append(xt00)
            wk8r = wk8.rearrange("t p j e -> p t j e")
            wv8r = wv8.rearrange("t p j e -> p t j e")
            nc.sync.dma_start(wk_sb[:, 1:4], wk8r[:, 1:4])
            nc.sync.dma_start(wk_sb[:, 4:8], wk8r[:, 4:8])
            for t in range(NT):
                if t > 0:
                    xt = xs_pool.tile([128, 2, 512], f8, tag="xs", name=f"xs0_{t}")
                    nc.sync.dma_start(xt[:], xT8[t, :, :, 0:512])
                    xts0.append(xt)
                nc.sync.dma_start(wq_sb[:, t], wq8[t])
                if t % 4 == 0:
                    nc.sync.dma_start(wv_sb[:, t : t + 4], wv8r[:, t : t + 4])
        elif bigdma:
            NT = DT
            xT, wq = aps["xT"], aps["wq"]
            wk, wv = aps["wk"], aps["wv"]
            wq_sb = wpool.tile([128, DT, EQ], f32r, tag="wq")
            wk_sb = wpool.tile([128, DT, HD], f32r, tag="wk")
            wv_sb = wpool.tile([128, DT, HD], f32r, tag="wv")
            xTr = xT.rearrange("(t p) s -> p t s", p=128)
            wqr = wq.rearrange("(t p) e -> p t e", p=128)
            wkr = wk.rearrange("(t p) e -> p t e", p=128)
            wvr = wv.rearrange("(t p) e -> p t e", p=128)
            # 4 d-slices per DMA; the first group is split so the very
            # first matmul ungates on a small load
            xg0 = xs_pool.tile([128, 4, 512], f32r, tag="xs4", bufs=6,
                               name="xg0_0")
            nc.sync.dma_start(wk_sb[:, 0:4, :], wkr[:, 0:4, :])
            nc.sync.dma_start(xg0[:, 0, 0:256], xTr[:, 0, 0:256])
            nc.sync.dma_start(xg0[:, 0, 256:512], xTr[:, 0, 256:512])
            nc.sync.dma_start(xg0[:, 1:4, :], xTr[:, 1:4, 0:512])
            xts0.append(xg0)
            for t in range(1, 4):
                xg = xs_pool.tile([128, 4, 512], f32r, tag="xs4", bufs=6,
                                  name=f"xg0_{t}")
                nc.sync.dma_start(xg[:], xTr[:, 4 * t : 4 * t + 4, 0:512])
                xts0.append(xg)
                nc.sync.dma_start(
                    wq_sb[:, 4 * (t - 1) : 4 * t, :], wqr[:, 4 * (t - 1) : 4 * t, :])
            nc.sync.dma_start(wq_sb[:, 12:16, :], wqr[:, 12:16, :])
            nc.sync.dma_start(wk_sb[:, 4:16, :], wkr[:, 4:16, :])
            nc.sync.dma_start(wv_sb[:, 0:16, :], wvr[:, 0:16, :])
        else:
            NT = DT
            xT, wq = aps["xT"], aps["wq"]
            wk, wv = aps["wk"], aps["wv"]
            wq_sb = wpool.tile([128, DT, EQ], f32r, tag="wq")
            wk_sb = wpool.tile([128, DT, HD], f32r, tag="wk")
            wv_sb = wpool.tile([128, DT, HD], f32r, tag="wv")
            xt00 = xs_pool.tile([128, 512], f32r, tag="xs", name="xs0_0")
            nc.sync.dma_start(wk_sb[:, 0, :], wk[0:128, :])
            nc.sync.dma_start(xt00[:, 0:256], xT[0:128, 0:256])
            nc.sync.dma_start(xt00[:, 256:512], xT[0:128, 256:512])
            xts0.append(xt00)
            wkr = wk.rearrange("(t p) e -> p t e", p=128)
            wvr = wv.rearrange("(t p) e -> p t e", p=128)
            nc.sync.dma_start(wk_sb[:, 1:4, :], wkr[:, 1:4, :])
            for d in range(DT):
                if d > 0:
                    xt = xs_pool.tile([128, 512], f32r, tag="xs", name=f"xs0_{d}")
                    nc.sync.dma_start(xt[:], xT[d * 128 : (d + 1) * 128, 0:512])
                    xts0.append(xt)
                nc.sync.dma_start(wq_sb[:, d, :], wq[d * 128 : (d + 1) * 128, :])
                if d % 4 == 0:
                    if d > 0:
                        nc.sync.dma_start(wk_sb[:, d : d + 4, :], wkr[:, d : d + 4, :])
                    nc.sync.dma_start(wv_sb[:, d : d + 4, :], wvr[:, d : d + 4, :])
        cc_sb = cspool.tile([128, S], f32, tag="cc")
        ss_sb = cspool.tile([128, S], f32, tag="ss")
        nc.sync.dma_start(cc_sb[:], ccT)
        nc.sync.dma_start(ss_sb[:], ssT)
        vT_sb = vt_pool.tile([128, S], bf16, tag="vT")

        for sc in range(SC):
            ssl = slice(sc * 512, (sc + 1) * 512)
            xts = xts0 if sc == 0 else None
            # Per-target accumulation loops (kp, qp0..qp3, vp) with RoPE
            # emitted right after each target completes: the RoPE pipeline
            # runs ~one target behind the matmuls instead of all landing
            # after the chunk, which shrinks the projection->attention
            # seam (the attention PSUM pools wait on this pool's releases).
            c = cc_sb[:, ssl]
            s = ss_sb[:, ssl]

            def _rope(src, dst):
                # dst[0:64]  = x0*c - x1*s ;  dst[64:128] = x0*s + x1*c
                # via two full-128-partition products: t_cc = src*[c;c] into
                # SBUF, then src *= [s;s] in place (PSUM). The cross-half
                # add/sub then mixes one SBUF and one PSUM operand — the
                # same-base-partition rule only binds when BOTH inputs are
                # in SBUF (NCC_IBIR297).
                if newrope:
                    t_cc = tmp_pool.tile([128, 512], f32, tag="tcc", name="tcc")
                    nc.vector.tensor_mul(t_cc[:], src[:], c)
                    nc.vector.tensor_mul(src[:], src[:], s)
                    nc.vector.tensor_sub(dst[0:64, :], t_cc[0:64, :], src[64:128, :])
                    nc.vector.tensor_add(dst[64:128, :], src[0:64, :], t_cc[64:128, :])
                    return
                t1 = tmp_pool.tile([64, 512], f32, tag="t1", name="t1")
                t2 = tmp_pool.tile([64, 512], f32, tag="t2", name="t2")
                t3 = tmp_pool.tile([64, 512], f32, tag="t3", name="t3")
                t4 = tmp_pool.tile([64, 512], f32, tag="t4", name="t4")
                nc.vector.tensor_mul(t1[:], src[0:64, :], c[0:64, :])
                nc.vector.tensor_mul(t2[:], src[64:128, :], s[0:64, :])
                nc.vector.tensor_mul(t3[:], src[0:64, :], s[0:64, :])
                nc.vector.tensor_mul(t4[:], src[64:128, :], c[0:64, :])
                nc.vector.tensor_sub(dst[0:64, :], t1[:], t2[:])
                nc.vector.tensor_add(dst[64:128, :], t3[:], t4[:])

            kp = mpsum.tile([128, 512], f32, tag="kv", bufs=2, name="kp")
            vp = mpsum.tile([128, 512], f32, tag="kv", bufs=2, name="vp")
            qps = [
                mpsum.tile([128, 512], f32, tag="qp", bufs=4, name=f"qp{h}")
                for h in range(HPC)
            ]
            # d-outer: each x tile is consumed by all 6 targets right away,
            # so the xs pool only needs ~8 rotating buffers (prefetch depth)
            # instead of a whole chunk of 16 live tiles.
            for d in range(NT):
                if xts is not None:
                    xt = xts[d // 4][:, d % 4, :] if bigdma else xts[d]
                elif bigdma:
                    if d % 4 == 0:
                        xg = xs_pool.tile([128, 4, 512], f32r, tag="xs4",
                                          bufs=6, name=f"xg{sc}_{d // 4}")
                        nc.sync.dma_start(xg[:], xTr[:, d : d + 4, ssl])
                    xt = xg[:, d % 4, :]
                elif fp8:
                    xt = xs_pool.tile([128, 2, 512], f8, tag="xs",
                                      name=f"xs{sc}_{d}")
                    nc.sync.dma_start(xt[:], xT8[d, :, :, ssl])
                else:
                    xt = xs_pool.tile([128, 512], f32r, tag="xs",
                                      name=f"xs{sc}_{d}")
                    nc.sync.dma_start(xt[:], xT[d * 128 : (d + 1) * 128, ssl])
                first = d == 0
                last = d == NT - 1
                if sc == 0 and d == 0:
                    # split so the very first matmul ungates on a 256-col load
                    if fp8:
                        nc.tensor.matmul(kp[:, 0:256], wk_sb[:, 0],
                                         xt[:, :, 0:256], start=True,
                                         stop=False, perf_mode=DR)
                        nc.tensor.matmul(kp[:, 256:512], wk_sb[:, 0],
                                         xt[:, :, 256:512], start=False,
                                         stop=False, perf_mode=DR)
                    else:
                        nc.tensor.matmul(kp[:, 0:256], wk_sb[:, 0, :],
                                         xt[:, 0:256], start=True, stop=False)
                        nc.tensor.matmul(kp[:, 256:512], wk_sb[:, 0, :],
                                         xt[:, 256:512], start=False,
                                         stop=False)
                elif fp8:
                    nc.tensor.matmul(kp[:], wk_sb[:, d], xt[:], start=first,
                                     stop=last, perf_mode=DR)
                else:
                    nc.tensor.matmul(kp[:], wk_sb[:, d, :], xt[:],
                                     start=first, stop=last)
                if fp8:
                    nc.tensor.matmul(vp[:], wv_sb[:, d], xt[:], start=first,
                                     stop=last, perf_mode=DR)
                else:
                    nc.tensor.matmul(vp[:], wv_sb[:, d, :], xt[:],
                                     start=first, stop=last)
                for h in range(HPC):
                    if fp8:
                        nc.tensor.matmul(
                            qps[h][:], wq_sb[:, d, :, h * 128 : (h + 1) * 128],
                            xt[:], start=first, stop=last, perf_mode=DR)
                    else:
                        nc.tensor.matmul(
                            qps[h][:], wq_sb[:, d, h * 128 : (h + 1) * 128],
                            xt[:], start=first, stop=last)
            # chunk tail: K rope first (frees its kv buf for the next chunk
            # while the PE does the V transposes), V evac on Scalar, Q ropes
            _rope(kp, krot[sc])
            nc.scalar.copy(vT_sb[:, ssl], vp[:])
            for h in range(HPC):
                _rope(qps[h], qrot[(h, sc)])
            for t in range(4 * sc, 4 * sc + 4):
                vtp = mpsum.tile([128, 128], f32, tag="vtr", bufs=2, name="vtr")
                nc.tensor.transpose(
                    vtp[:], vT_sb[:, t * 128 : (t + 1) * 128], ident[:]
                )
                nc.scalar.copy(vsb[:, t, :], vtp[:])

    # ---------------- Phase C: attention per (head, q-chunk) --------
    # Flattened tile list, software-pipelined depth 2: the score matmul for
    # tile i+2 is emitted before the PV/denom matmuls of tile i so the PE
    # has ~1.3us of queued work while the Scalar exp of tile i completes.
    with ExitStack() as stkC:
        if pools is None:
            opool = stkC.enter_context(tc.tile_pool(name="opool", bufs=1))
            wopool = stkC.enter_context(tc.tile_pool(name="wopool", bufs=1))
        else:
            opool, wopool = pools["opool"], pools["wopool"]
        orot = {
            (h, c): opool.tile(
                [128, 512], bf16, tag=f"orot{h}_{c}", name=f"orot{h}_{c}"
            )
            for h in range(HPC)
            for c in range(QC)
        }
        wo_sb = wopool.tile([128, HPC, D], bf16, tag="wo")
        for h in range(HPC):
            nc.sync.dma_start(wo_sb[:, h, :], wo16[h * 128 : (h + 1) * 128, :])

        with ExitStack() as stkE:
            if pools is None:
                et_pool = stkE.enter_context(tc.tile_pool(name="et", bufs=12))
                rd_pool = stkE.enter_context(tc.tile_pool(name="rd", bufs=2))
            else:
                et_pool, rd_pool = pools["et"], pools["rd"]
            tiles = []
            for h in range(HPC):
                for qc in range(QC):
                    nkt = 4 * (qc + 1)
                    for kt in range(nkt):
                        tiles.append((h, qc, kt, nkt))
            ntile = len(tiles)

            def _lo(qc, kt):
                r = kt - 4 * qc
                if not trim or r <= 0:
                    return 0, r
                if biasmask and r == 3:
                    return 256, r
                return 128 * r, r

            acc = {}  # (h, qc) -> (op, dp) PSUM accumulators

            def emit_S(i):
                h, qc, kt, nkt = tiles[i]
                lo, r = _lo(qc, kt)
                sp = mpsum.tile([128, 512], f32, tag="qp", bufs=4, name="sp")
                kc, ko = divmod(kt, 4)
                diag = r >= 0 and biasmask
                nc.tensor.matmul(
                    sp[:, lo:512],
                    krot[kc][:, ko * 128 : (ko + 1) * 128],
                    qrot[(h, qc)][:, lo:512],
                    start=True,
                    stop=not diag,
                )
                if diag:
                    # -1e9 bias on masked (k > q) entries, accumulated in
                    # PSUM: out[p,m] += sum_k (-1e9*I)[k,p] * tri01[k,m].
                    if r == 3:
                        nc.tensor.matmul(
                            sp[:, 256:512], negI_bf[:], triB_bf[:, 0:256],
                            start=False, stop=True,
                        )
                    else:
                        nc.tensor.matmul(
                            sp[:, 128 * r : 128 * r + 128],
                            negI_bf[:], triA_bf[:, 0:128],
                            start=False, stop=True,
                        )
                return sp

            def emit_E(i, sp):
                h, qc, kt, nkt = tiles[i]
                lo, r = _lo(qc, kt)
                et = et_pool.tile([128, 512], bf16, tag="et", name="et")
                nc.scalar.activation(
                    et[:, lo:512], sp[:, lo:512], AT.Exp, scale=escale
                )
                if r >= 0 and not biasmask:
                    nc.vector.tensor_mul(
                        et[:, lo : lo + 128], et[:, lo : lo + 128], masks_sb[:]
                    )
                return et

            def emit_A(i, et):
                h, qc, kt, nkt = tiles[i]
                lo, _ = _lo(qc, kt)
                if kt == 0:
                    op = mpsum.tile([128, 512], f32, tag="vtr", bufs=2, name="op")
                    dp = mpsum.tile([128, 512], f32, tag="kv", bufs=2, name="dp")
                    da = None
                    if dvedp:
                        da = rd_pool.tile([128, 512], bf16, tag="dacc",
                                          name="dacc")
                    acc[(h, qc)] = (op, dp, da)
                op, dp, da = acc[(h, qc)]
                st = kt == 0
                spf = kt == nkt - 1
                nc.tensor.matmul(
                    op[:, lo:512], vsb[:, kt, :], et[:, lo:512], start=st, stop=spf
                )
                if dvedp:
                    # bf16 partial sums on the DVE (2x/4x packed modes);
                    # the 128-partition reduction happens once per group in
                    # the single ones-matmul below
                    if st:
                        nc.vector.tensor_copy(da[:], et[:])
                    else:
                        nc.vector.tensor_add(
                            da[:, lo:512], da[:, lo:512], et[:, lo:512])
                else:
                    nc.tensor.matmul(
                        dp[:, lo:512], ones_sb[:], et[:, lo:512], start=st,
                        stop=spf)
                if spf:
                    if dvedp:
                        nc.tensor.matmul(dp[:], ones_sb[:], da[:], start=True,
                                         stop=True)
                    rd = rd_pool.tile([128, 512], f32, tag="rd", name="rd")
                    nc.vector.reciprocal_approx_fast(rd[:], dp[:])
                    nc.vector.tensor_mul(orot[(h, qc)][:], op[:], rd[:])

            if pipe:
                sps = {0: emit_S(0)}
                if ntile > 1:
                    sps[1] = emit_S(1)
                ets = {0: emit_E(0, sps[0])}
                for i in range(ntile):
                    if i + 2 < ntile:
                        sps[i + 2] = emit_S(i + 2)
                    emit_A(i, ets.pop(i))
                    if i + 1 < ntile:
                        ets[i + 1] = emit_E(i + 1, sps.pop(i + 1))
            else:
                for i in range(ntile):
                    emit_A(i, emit_E(i, emit_S(i)))

        # ---------------- Phase D: output projection ----------------
        with ExitStack() as stkD:
            if pools is None:
                ostage_pool = stkD.enter_context(
                    tc.tile_pool(name="ostage", bufs=6))
            else:
                ostage_pool = pools["ostage"]
            for st in range(16):
                stsl = slice(st * 128, (st + 1) * 128)
                if bigdma:
                    ostw = ostage_pool.tile([128, 2048], f32, tag="ost4",
                                            bufs=2, name="ostw")
                for mc in range(4):
                    msl = slice(mc * 512, (mc + 1) * 512)
                    pout = mpsum.tile([128, 512], f32, tag="qp", bufs=4, name="pout")
                    sc_, so = divmod(st, 4)
                    for h in range(HPC):
                        nc.tensor.matmul(
                            pout[:],
                            orot[(h, sc_)][:, so * 128 : (so + 1) * 128],
                            wo_sb[:, h, msl],
                            start=(h == 0),
                            stop=(h == HPC - 1),
                        )
                    if bigdma:
                        ost = ostw[:, msl]
                    else:
                        ost = ostage_pool.tile(
                            [128, 512], f32, tag="ost", name="ost")
                    # alternate evac engines so neither gates the PE stream;
                    # under fp8 the V-path carries a W8SCALE factor — divide
                    # it out here (scaled copy costs the same as copy)
                    dsc = 1.0 / W8SCALE if fp8 else 1.0
                    if evacalt and (st * 4 + mc) % 2 == 1:
                        if fp8:
                            nc.vector.tensor_scalar_mul(ost[:], pout[:], dsc)
                        else:
                            nc.vector.tensor_copy(ost[:], pout[:])
                    elif fp8:
                        nc.scalar.mul(ost[:], pout[:], dsc)
                    else:
                        nc.scalar.copy(ost[:], pout[:])
                    if not bigdma:
                        nc.sync.dma_start(out[stsl, msl], ost[:])
                if bigdma:
                    nc.gpsimd.dma_start(out[stsl, :], ostw[:])

    if mpsum_ctx is not None:
        mpsum_ctx.__exit__(None, None, None)


def _build_program(reps: int = 1, trim: bool = True, split: bool = True,
                   pipe: bool = True, biasmask: bool = True,
                   newrope: bool = True, evacalt: bool = True,
                   fp8: bool = False, hoist: bool = False,
                   bigdma: bool = False, dvedp: bool = False,
                   mergecd: bool = False, w16: bool = False, dlag: int = 2,
                   pairexp: bool = False, pooldp: bool = False,
                   dvemask: bool = False, dacc2: bool = False):
    from contextlib import ExitStack

    import concourse.mybir as mybir
    import concourse.tile as tile
    from concourse import bacc
    from concourse.masks import make_identity


    f32 = mybir.dt.float32
    f32r = mybir.dt.float32r
    bf16 = mybir.dt.bfloat16
    f8 = mybir.dt.float8e4

    nc = bacc.Bacc(
        "TRN2",
        target_bir_lowering=False,
        debug=False,
        enable_asserts=True,
        num_devices=N_CORES,
    )

    aps = {}
    if fp8:
        aps["xT8"] = nc.dram_tensor(
            "xT8", [DT // 2, 128, 2, S], f8, kind="ExternalInput").ap()
        aps["wq8"] = nc.dram_tensor(
            "wq8", [DT // 2, 128, 2, EQ], f8, kind="ExternalInput").ap()
        aps["wk8"] = nc.dram_tensor(
            "wk8", [DT // 2, 128, 2, HD], f8, kind="ExternalInput").ap()
        aps["wv8"] = nc.dram_tensor(
            "wv8", [DT // 2, 128, 2, HD], f8, kind="ExternalInput").ap()
    elif not (mergecd and w16):
        aps["xT"] = nc.dram_tensor("xT", [D, S], f32r, kind="ExternalInput").ap()
        aps["wq"] = nc.dram_tensor("wq", [D, EQ], f32r, kind="ExternalInput").ap()
        aps["wk"] = nc.dram_tensor("wk", [D, HD], f32r, kind="ExternalInput").ap()
        aps["wv"] = nc.dram_tensor("wv", [D, HD], f32r, kind="ExternalInput").ap()
    aps["wo16"] = nc.dram_tensor(
        "wo16", [EQ, D], mybir.dt.bfloat16, kind="ExternalInput").ap()
    if mergecd and w16:
        aps["xT16"] = nc.dram_tensor("xT16", [D, S], bf16,
                                     kind="ExternalInput").ap()
        aps["wq16"] = nc.dram_tensor("wq16", [D, EQ], bf16,
                                     kind="ExternalInput").ap()
        aps["wk16"] = nc.dram_tensor("wk16", [D, HD], bf16,
                                     kind="ExternalInput").ap()
        aps["wv16"] = nc.dram_tensor("wv16", [D, HD], bf16,
                                     kind="ExternalInput").ap()
    aps["ccT"] = nc.dram_tensor("ccT", [128, S], f32, kind="ExternalInput").ap()
    aps["ssT"] = nc.dram_tensor("ssT", [128, S], f32, kind="ExternalInput").ap()
    triA = nc.dram_tensor("triA", [128, 512], f32, kind="ExternalInput").ap()
    triB = nc.dram_tensor("triB", [128, 256], f32, kind="ExternalInput").ap()
    aps["out"] = nc.dram_tensor("out", [S, D], f32, kind="ExternalOutput").ap()

    with tile.TileContext(nc) as tc:
        with (
            tc.tile_pool(name="persist", bufs=1) as persist,
            tc.tile_pool(name="consts", bufs=1) as consts,
        ):
            qrot = {
                (h, c): persist.tile(
                    [128, 512], f32r, tag=f"qrot{h}_{c}", name=f"qrot{h}_{c}"
                )
                for h in range(HPC)
                for c in range(SC)
            }
            krot = {
                c: persist.tile([128, 512], f32r, tag=f"krot{c}", name=f"krot{c}")
                for c in range(SC)
            }
            vsb = persist.tile([128, KT, HD], bf16, tag="vsb")

            ones_f32 = consts.tile([128, 128], f32, tag="ones_f32")
            nc.gpsimd.memset(ones_f32[:], 1.0)
            ones_sb = consts.tile([128, 128], bf16, tag="ones")
            nc.vector.tensor_copy(ones_sb[:], ones_f32[:])
            ident = consts.tile([128, 128], f32, tag="ident")
            make_identity(nc, ident[:])
            ident_bf = consts.tile([128, 128], bf16, tag="ident_bf")
            nc.vector.tensor_copy(ident_bf[:], ident[:])
            negI_bf = consts.tile([128, 128], bf16, tag="negI")
            nc.vector.tensor_scalar_mul(negI_bf[:], ident[:], -1e9)
            triA_f = consts.tile([128, 512], f32, tag="triA_f")
            nc.sync.dma_start(triA_f[:], triA)
            triA_bf = consts.tile([128, 512], bf16, tag="triA")
            nc.vector.tensor_copy(triA_bf[:], triA_f[:])
            triB_f = consts.tile([128, 256], f32, tag="triB_f")
            nc.sync.dma_start(triB_f[:], triB)
            triB_bf = consts.tile([128, 256], bf16, tag="triB")
            nc.vector.tensor_copy(triB_bf[:], triB_f[:])
            # multiplicative keep-mask (1 - tri) for the biasmask=False path
            masks_sb = consts.tile([128, 128], f32, tag="masks")
            nc.vector.tensor_sub(masks_sb[:], ones_f32[:], triA_f[:, 0:128])
            # bf16 keep-masks for the dvemask path: [128,128] (1-tri) and
            # [128,256] (zeros | 1-tri) for the padded r=3 tile
            masks_bf = consts.tile([128, 128], bf16, tag="masks_bf")
            nc.vector.tensor_copy(masks_bf[:], masks_sb[:])
            masksB_bf = consts.tile([128, 256], bf16, tag="masksB_bf")
            nc.gpsimd.memset(masksB_bf[:, 0:128], 0.0)
            nc.vector.tensor_copy(masksB_bf[:, 128:256], masks_sb[:])

            shared = (ones_sb, ident, negI_bf, triA_bf, triB_bf, masks_sb,
                      qrot, krot, vsb, ident_bf, masks_bf, masksB_bf)
            with ExitStack() as stk:
                pools = None
                if hoist:
                    mk = lambda name, bufs, **kw: stk.enter_context(
                        tc.tile_pool(name=name, bufs=bufs, **kw))
                    pools = {
                        "mpsum": mk("mpsum", 1, space="PSUM"),
                        "wpool": mk("wpool", 1),
                        "cspool": mk("cspool", 1),
                        "xs": mk("xs", 8),
                        "ropetmp": mk("ropetmp", 2),
                        "vtstage": mk("vtstage", 1),
                        "opool": mk("opool", 1),
                        "wopool": mk("wopool", 1),
                        "et": mk("et", 8),

                        "rd": mk("rd", 2),
                        "ostage": mk("ostage", 4),
                    }
                for _rep in range(reps):
                    if mergecd:
                        _emit_fused(nc, tc, mybir, aps, shared,
                                    evacalt=evacalt, newrope=newrope,
                                    w16=w16, dlag=dlag, pairexp=pairexp,
                                    pooldp=pooldp, dvemask=dvemask,
                                    dacc2=dacc2)
                    else:
                        _emit_once(nc, tc, mybir, aps, shared, trim=trim,
                                   split=split, pipe=pipe, biasmask=biasmask,
                                   newrope=newrope, evacalt=evacalt, fp8=fp8,
                                   bigdma=bigdma, dvedp=dvedp, pools=pools)

    nc.compile()
    return nc


def _make_in_maps(x, Wq, Wk, Wv, Wo):
    cc, ss, perm = _host_tables()
    triA, triB = _host_tris()
    x = np.asarray(x, np.float32)
    Wq = np.asarray(Wq, np.float32)
    Wk = np.asarray(Wk, np.float32)
    Wv = np.asarray(Wv, np.float32)
    Wo = np.asarray(Wo, np.float32)

    # per-head column deinterleave for RoPE half-form
    qperm = np.concatenate([h * HD + perm for h in range(N_HEADS)])
    kperm = np.concatenate([h * HD + perm for h in range(N_KV_HEADS)])
    Wqp = Wq[:, qperm]
    Wkp = Wk[:, kperm]

    import concourse.mybir as mybir

    e4 = mybir.dt.np(mybir.dt.float8e4)
    bf16np = mybir.dt.np(mybir.dt.bfloat16)

    def pack8(a2d, scale):
        # [D, M] -> [D/256, 128, 2, M]; row d = 256t + 128j + p
        a = (np.asarray(a2d, np.float32) * scale).reshape(
            DT // 2, 2, 128, a2d.shape[1]
        )
        return np.ascontiguousarray(a.transpose(0, 2, 1, 3)).astype(e4)

    w16 = FLAGS.get("mergecd") and FLAGS.get("w16")
    fp8 = FLAGS.get("fp8")
    in_maps = []
    for core in range(N_CORES):
        b, g = divmod(core, G)
        # each program variant binds only the names it declares; build only
        # the forms the active FLAGS need (pack8 of 16MB x 8 cores is slow)
        m = {
            "wo16": np.ascontiguousarray(
                Wo[g * EQ : (g + 1) * EQ, :]).astype(bf16np),
            "ccT": cc,
            "ssT": ss,
            "triA": triA,
            "triB": triB,
        }
        if w16:
            m.update({
                "xT16": np.ascontiguousarray(x[b].T).astype(bf16np),
                "wq16": np.ascontiguousarray(
                    Wqp[:, g * EQ : (g + 1) * EQ]).astype(bf16np),
                "wk16": np.ascontiguousarray(
                    Wkp[:, g * HD : (g + 1) * HD]).astype(bf16np),
                "wv16": np.ascontiguousarray(
                    Wv[:, g * HD : (g + 1) * HD]).astype(bf16np),
            })
        else:
            m.update({
                "xT": np.ascontiguousarray(x[b].T),
                "wq": np.ascontiguousarray(Wqp[:, g * EQ : (g + 1) * EQ]),
                "wk": np.ascontiguousarray(Wkp[:, g * HD : (g + 1) * HD]),
                "wv": np.ascontiguousarray(Wv[:, g * HD : (g + 1) * HD]),
            })
        if fp8:
            m.update({
                "xT8": pack8(x[b].T, 1.0),
                "wq8": pack8(Wqp[:, g * EQ : (g + 1) * EQ], W8SCALE),
                "wk8": pack8(Wkp[:, g * HD : (g + 1) * HD], W8SCALE),
                "wv8": pack8(Wv[:, g * HD : (g + 1) * HD], W8SCALE),
            })
        in_maps.append(m)
    return in_maps


_CACHE = {}


def _get_program(reps: int = 1, trim: bool = True, split: bool = True,
                 pipe: bool = True, biasmask: bool = True,
                 newrope: bool = True, evacalt: bool = True,
                 fp8: bool = False, hoist: bool = False,
                 bigdma: bool = False, dvedp: bool = False,
                 mergecd: bool = False, w16: bool = False, dlag: int = 2,
                 pairexp: bool = False, pooldp: bool = False,
                 dvemask: bool = False, dacc2: bool = False):
    key = ("nc", reps, trim, split, pipe, biasmask, newrope, evacalt, fp8,
           hoist, bigdma, dvedp, mergecd, w16, dlag, pairexp, pooldp,
           dvemask, dacc2)
    if key not in _CACHE:
        _CACHE[key] = _build_program(
            reps, trim=trim, split=split, pipe=pipe, biasmask=biasmask,
            newrope=newrope, evacalt=evacalt, fp8=fp8, hoist=hoist,
            bigdma=bigdma, dvedp=dvedp, mergecd=mergecd, w16=w16, dlag=dlag,
            pairexp=pairexp, pooldp=pooldp, dvemask=dvemask, dacc2=dacc2,
        )
    return _CACHE[key]


def kernel(x, mask, Wq, Wk, Wv, Wo):
    from concourse.bass_utils import run_bass_kernel_spmd

    nc = _get_program(**FLAGS)
    in_maps = _make_in_maps(x, Wq, Wk, Wv, Wo)
    res = run_bass_kernel_spmd(nc, in_maps, core_ids=list(range(N_CORES)))
    parts = [res.results[c]["out"] for c in range(N_CORES)]
    out = np.stack(
        [
            parts[0] + parts[1] + parts[2] + parts[3],
            parts[4] + parts[5] + parts[6] + parts[7],
        ]
    ).astype(np.float32)
    return out

